# revision 1
# baseline (speedup 1.0000x reference)
"""Trainium2 Bass kernel for a 2-layer GRU (B=4096, T=128, D=32, H=64) + linear head.

Strategy
--------
Data-parallel over batch: B=4096 -> 8 NeuronCores x 512. Each core runs the
full T=128 recurrence for its batch shard. Layout on chip is "gate-major":
activations live as [gates/hidden on partitions, batch on the free dim], so
the recurrent matmuls are `W^T (stationary) x state (moving)` with N=512
streamed columns and all elementwise work has free-dim 512.

Per GRU step (layer l, input src [K,512], state [64,512]):
  psum_rz[128,512]  = Wx_rz^T src + Wh_rz^T state          (PE, accumulated)
  rz                = sigmoid(psum_rz + b_rz)              (ACT, bias folded)
  t                 = (psum_hn + b_hh_n) * r               (DVE scalar_tensor_tensor)
  psum_xn          += I64 @ t                              (PE identity-accumulate)
  n                 = tanh(psum_xn + b_ih_n)               (ACT, bias folded)
  d = state - n     (GPSIMD)   e = z*d  (DVE)   state' = n + e  (DVE)

The two GRU layers are pipelined one step apart (wavefront), so the
sequential per-step dependency chain of one layer overlaps with the other
layer's work on every engine.
"""

import sys

if "/opt/trn_rl_repo" not in sys.path:
    sys.path.insert(0, "/opt/trn_rl_repo")

import numpy as np
import ml_dtypes

B, T, D, H = 4096, 128, 32, 64
NCORES = 8
BL = B // NCORES  # per-core batch = 512
G3 = 3 * H        # 192 gates, order [r | z | n]

_CACHE = {}


def _legalize_sync(nc, mybir):
    """Split per-instruction semaphore waits that exceed the ISA wait-slot
    budget into EventSemaphore instructions on the same engine queue.

    This walrus build enforces (empirically): compute engines (ACT/DVE/Pool)
    1 wait, DMA 2, PE matmul 3, Drain/CTRL 2. Tile's scheduler freely attaches
    more; excess waits are moved to wait-only EVSEMs issued immediately
    before, which the engine sequencer executes in order — identical
    semantics, legal encoding.
    """
    budget = {
    }  # every instruction type: 1 wait max (walrus adds internal waits)
    ctr = 0
    for f in nc.m.functions:
        for blk in f.blocks:
            out = []
            changed = False
            for inst in blk.instructions:
                si = inst.sync_info
                waits = list(si.on_wait) if (si is not None and si.on_wait) else []
                b = budget.get(type(inst).__name__, 1)
                if len(waits) > b:
                    excess, keep = waits[:-b], waits[-b:]
                    for w in excess:
                        ctr += 1
                        out.append(
                            mybir.InstEventSemaphore(
                                name=f"evw{ctr}_{inst.name}",
                                engine=inst.engine,
                                ins=[],
                                outs=[],
                                sync_info=mybir.SyncInfo(on_wait=[w], on_update=[]),
                            )
                        )
                    si.on_wait = keep
                    changed = True
                out.append(inst)
            if changed:
                try:
                    blk.instructions = out
                except Exception:
                    blk.instructions.clear()
                    blk.instructions.extend(out)
    return ctr


def build_module(t_steps=T, bl=BL, reps=1):
    """Build the Bass module (single program, run SPMD on 8 cores).

    reps>1 repeats the whole wavefront (same x) for slope-timing the real
    device execution under the ~80ms axon dispatch overhead.
    """
    from contextlib import ExitStack

    import concourse.bass as bass
    import concourse.tile as tile
    from concourse import mybir

    f32 = mybir.dt.float32
    bf16 = mybir.dt.bfloat16
    AF = mybir.ActivationFunctionType
    OP = mybir.AluOpType

    nc = bass.Bass()

    # ---- DRAM I/O (per-core shapes) ----
    # All small constants are host-packed into two tensors so they arrive in
    # two DMAs (one semaphore source each) — per-instruction wait slots are a
    # scarce HW resource (setupSyncWait limit).
    CW = 840  # bf16 const pack width
    x_d = nc.dram_tensor("x", [t_steps, D, bl], bf16, kind="ExternalInput")
    cb_d = nc.dram_tensor("cb", [128, CW], bf16, kind="ExternalInput")
    cf_d = nc.dram_tensor("cf", [128, 8], f32, kind="ExternalInput")
    out_d = nc.dram_tensor("out", [1, bl], f32, kind="ExternalOutput")

    with ExitStack() as ctx:
        tc = ctx.enter_context(tile.TileContext(nc))
        const = ctx.enter_context(tc.tile_pool(name="const", bufs=1))
        xpool = ctx.enter_context(tc.tile_pool(name="xp", bufs=6))
        spool = ctx.enter_context(tc.tile_pool(name="state", bufs=8))
        work = ctx.enter_context(tc.tile_pool(name="work", bufs=8))
        ps_rz = ctx.enter_context(tc.tile_pool(name="ps_rz", bufs=2, space="PSUM"))
        ps_xn = ctx.enter_context(tc.tile_pool(name="ps_xn", bufs=3, space="PSUM"))
        ps_hn = ctx.enter_context(tc.tile_pool(name="ps_hn", bufs=3, space="PSUM"))

        # ---- constants in SBUF (two packed tiles, two DMAs) ----
        cb = const.tile([128, CW], bf16, tag="cb")
        nc.sync.dma_start(out=cb, in_=cb_d[:])
        cf = const.tile([128, 8], f32, tag="cf")
        nc.sync.dma_start(out=cf, in_=cf_d[:])
        wx_sb = [cb[0:D, 0:G3], cb[0:H, G3 : 2 * G3]]
        wh_sb = [cb[0:H, 2 * G3 : 3 * G3], cb[0:H, 3 * G3 : 4 * G3]]
        ident = cb[:, 4 * G3 : 4 * G3 + H]  # identity at partitions 64:128
        fcw_sb = cb[0:H, 4 * G3 + H : 4 * G3 + H + 1]
        brz_sb = [cf[:, 0:1], cf[:, 1:2]]
        bni_sb = [cf[0:H, 2:3], cf[0:H, 3:4]]
        bnh_sb = [cf[:, 4:5], cf[:, 5:6]]
        fcb_sb = cf[0:1, 6:7]

        # ACT warm-up: absorbs the sigmoid/tanh table-load and the cf DMA
        # wait into an instruction with spare wait slots (ACT wait-slot limit).
        warm = work.tile([128, 8], f32, tag="warm")
        nc.scalar.activation(warm, cf, AF.Sigmoid)
        warm_v = work.tile([128, 8], f32, tag="warm_v")
        nc.vector.tensor_copy(warm_v, cf)

        # Preload all of x: 8 chunk tiles written once each (no WAR/WAW waits
        # on the hot path; consumers wait on one DMA sem per 16 steps).
        CH = max(1, t_steps // 8)
        x_chunks = []
        for c in range(0, t_steps, CH):
            n_t = min(CH, t_steps - c)
            xc = const.tile([D, n_t, bl], bf16, tag=f"xc{c}")
            nc.sync.dma_start(
                out=xc, in_=x_d[c : c + n_t].rearrange("t d b -> d t b")
            )
            x_chunks.append(xc)

        def x_slice(s):
            return x_chunks[s // CH][:, s % CH, :]

        def gru_step(l, src, state_prev):
            """Emit one GRU step; returns the new state tile [H, bl] bf16.

            Gate order is [z | r | n] (host pre-permuted): z at partitions
            0:64 aligns with the h-space tensors (state/n/d/e, base 0);
            r at partitions 64:128 aligns with hn/t (base 64), so every
            SBUF-SBUF tensor_tensor has equal start partitions.
            """
            prz = ps_rz.tile([2 * H, bl], f32, tag="rz")
            nc.tensor.matmul(prz, lhsT=wx_sb[l][:, 0 : 2 * H], rhs=src,
                             start=True, stop=False)
            nc.tensor.matmul(prz, lhsT=wh_sb[l][:, 0 : 2 * H], rhs=state_prev,
                             start=False, stop=True)
            # xn -> partitions 0:64 of its bank; hn -> partitions 64:128
            pxn = ps_xn.tile([2 * H, bl], f32, tag="xn")
            nc.tensor.matmul(pxn[0:H, :], lhsT=wx_sb[l][:, 2 * H : G3], rhs=src,
                             start=True, stop=False, skip_group_check=True)
            phn = ps_hn.tile([2 * H, bl], f32, tag="hn")
            nc.tensor.matmul(phn[H : 2 * H, :], lhsT=wh_sb[l][:, 2 * H : G3],
                             rhs=state_prev, start=True, stop=True)

            rz = work.tile([2 * H, bl], bf16, tag="rz_s")
            nc.scalar.activation(rz, prz, AF.Sigmoid, bias=brz_sb[l])

            # t = (hn + b_hh_n) * r   on lanes 64:128
            t = work.tile([2 * H, bl], bf16, tag="t")
            nc.vector.scalar_tensor_tensor(
                out=t[H : 2 * H, :], in0=phn[H : 2 * H, :],
                scalar=bnh_sb[l][H : 2 * H, :], in1=rz[H : 2 * H, :],
                op0=OP.add, op1=OP.mult)

            # psum_xn[0:64] += t  (identity stationary at rows 64:128)
            nc.tensor.matmul(pxn[0:H, :], lhsT=ident[H : 2 * H, :],
                             rhs=t[H : 2 * H, :], start=False, stop=True,
                             skip_group_check=True)

            n = work.tile([H, bl], bf16, tag="n")
            nc.scalar.activation(n, pxn[0:H, :], AF.Tanh, bias=bni_sb[l])

            d = work.tile([H, bl], bf16, tag="d")
            nc.gpsimd.tensor_sub(d, state_prev, n)
            e = work.tile([H, bl], bf16, tag="e")
            nc.vector.tensor_mul(e, rz[0:H, :], d)
            ns = spool.tile([H, bl], bf16, tag=("g" if l == 0 else "h"))
            nc.vector.tensor_add(ns, n, e)
            return ns

        g_prev = spool.tile([H, bl], bf16, tag="g")
        h_prev = spool.tile([H, bl], bf16, tag="h")
        nc.vector.memset(g_prev, 0.0)
        nc.vector.memset(h_prev, 0.0)

        n_steps = t_steps * reps
        g_list = [None] * (n_steps + 1)
        g_list[0] = g_prev  # g_list[s+1] = layer-0 output at step s

        for s in range(n_steps + 1):
            if s < n_steps:
                g_list[s + 1] = gru_step(0, x_slice(s % t_steps), g_list[s])
            if s >= 1:
                # layer 1, step s-1 consumes layer-0 output of step s-1
                h_prev = gru_step(1, g_list[s], h_prev)

        # final projection: out = fc_w @ h_T + fc_b   -> [1, bl]
        pfc = ps_rz.tile([1, bl], f32, tag="rz")
        nc.tensor.matmul(pfc, lhsT=fcw_sb, rhs=h_prev, start=True, stop=True)
        out_sb = work.tile([1, bl], f32, tag="out")
        nc.scalar.activation(out_sb, pfc, AF.Identity, bias=fcb_sb)
        nc.sync.dma_start(out=out_d[:], in_=out_sb)

    _legalize_sync(nc, mybir)
    return nc


def shard_inputs(inputs, bl=BL, ncores=NCORES, t_steps=T):
    """Host-side prep: transpose/cast/shard full inputs into per-core maps."""
    bf = ml_dtypes.bfloat16
    x = np.asarray(inputs["x"], dtype=np.float32)
    xT = np.ascontiguousarray(x[: bl * ncores, :t_steps, :].transpose(1, 2, 0)).astype(bf)

    def wT(w):
        return np.ascontiguousarray(np.asarray(w, dtype=np.float32).T).astype(bf)

    def gates_zrn(w):
        """Permute gate rows [r|z|n] -> [z|r|n], then transpose to [in, 3H]."""
        w = np.asarray(w, dtype=np.float32)
        w = np.concatenate([w[H : 2 * H], w[0:H], w[2 * H :]], axis=0)
        return np.ascontiguousarray(w.T).astype(bf)

    CW = 840
    cb = np.zeros((128, CW), dtype=bf)
    cb[0:D, 0:G3] = gates_zrn(inputs["W_ih0"])
    cb[0:H, G3 : 2 * G3] = gates_zrn(inputs["W_ih1"])
    cb[0:H, 2 * G3 : 3 * G3] = gates_zrn(inputs["W_hh0"])
    cb[0:H, 3 * G3 : 4 * G3] = gates_zrn(inputs["W_hh1"])
    cb[H:128, 4 * G3 : 4 * G3 + H] = np.eye(H, dtype=np.float32).astype(bf)
    cb[0:H, 4 * G3 + H] = wT(inputs["fc_w"]).reshape(H)

    cf = np.zeros((128, 8), dtype=np.float32)
    for l in range(2):
        bi = np.asarray(inputs[f"b_ih{l}"], dtype=np.float32)
        bh = np.asarray(inputs[f"b_hh{l}"], dtype=np.float32)
        bzr = bi[: 2 * H] + bh[: 2 * H]
        cf[:, l] = np.concatenate([bzr[H:], bzr[:H]])  # [z | r] order
        cf[0:H, 2 + l] = bi[2 * H :]
        cf[H:128, 4 + l] = bh[2 * H :]
    cf[0, 6] = np.asarray(inputs["fc_b"], dtype=np.float32).reshape(())

    shared = {"cb": cb, "cf": cf}

    in_maps = []
    for c in range(ncores):
        m = dict(shared)
        m["x"] = np.ascontiguousarray(xT[:, :, c * bl : (c + 1) * bl])
        in_maps.append(m)
    return in_maps


def kernel(**inputs):
    from concourse import bass_utils

    if "nc" not in _CACHE:
        _CACHE["nc"] = build_module()
    nc = _CACHE["nc"]
    in_maps = shard_inputs(inputs)
    res = bass_utils.run_bass_kernel_spmd(nc, in_maps, core_ids=list(range(NCORES)))
    out = np.concatenate([r["out"].reshape(BL) for r in res.results])
    return out.astype(np.float32)



# revision 6
# speedup vs baseline: 1.1259x; 1.1259x over previous
"""Trainium2 Bass kernel for a 2-layer GRU (B=4096, T=128, D=32, H=64) + linear head.

Strategy
--------
Data-parallel over batch: B=4096 -> 8 NeuronCores x 512. Each core runs the
full T=128 recurrence for its batch shard, gate-major on chip: gates/hidden
on partitions, batch on the free dim.

The two GRU layers run as a wavefront (layer 1 one step behind layer 0) and
are FUSED onto shared partition ranges: layer 0 owns partitions 0:64, layer 1
owns 64:128 of four cross-layer PSUM gate banks ([z0|z1], [r0|r1],
[hn0|hn1], [xn0|xn1]) and of the combined state tile C(s) = [h0(s); h1(s-1)].
Every sigmoid/tanh/DVE op then covers BOTH layers in one [128, fw]
instruction, and layer 1's z/r matmuls contract over the full C (K=128) in
one pass. The batch is further split into independent column streams so the
serial per-step dependency chains overlap on the engines.

Per wavefront step (per stream, free width fw):
  PE : r,z gates (l0: x-part + h-part accum; l1: one K=128 mm), hn, xn(start)
  ACT: R = sigmoid(rb + br)      [128,fw]  (both layers at once)
  DVE: T = (hb + bnh) * R        (scalar_tensor_tensor)
  PE : nb += I @ T               (identity accumulate onto xn)
  ACT: Z = sigmoid(zb + bz);  N = tanh(nb + bni)
  Pool: ZH = Z * C(s-1)          (off critical path)
  DVE: E3 = (Z - 1) * N;  C(s) = ZH - E3   # = z*h + (1-z)*n for both layers
"""

import sys

if "/opt/trn_rl_repo" not in sys.path:
    sys.path.insert(0, "/opt/trn_rl_repo")

import numpy as np
import ml_dtypes

B, T, D, H = 4096, 128, 32, 64
NCORES = 8
BL = B // NCORES  # per-core batch = 512
STREAMS = 2

_CACHE = {}


def _legalize_sync(nc, mybir):
    """Split per-instruction semaphore waits that exceed the ISA wait-slot
    budget into EventSemaphore instructions on the same engine queue."""
    budget = {}  # every instruction type: 1 wait max (walrus adds internal waits)
    ctr = 0
    for f in nc.m.functions:
        for blk in f.blocks:
            out = []
            changed = False
            for inst in blk.instructions:
                si = inst.sync_info
                waits = list(si.on_wait) if (si is not None and si.on_wait) else []
                b = budget.get(type(inst).__name__, 1)
                if len(waits) > b:
                    excess, keep = waits[:-b], waits[-b:]
                    for w in excess:
                        ctr += 1
                        out.append(
                            mybir.InstEventSemaphore(
                                name=f"evw{ctr}_{inst.name}",
                                engine=inst.engine,
                                ins=[],
                                outs=[],
                                sync_info=mybir.SyncInfo(on_wait=[w], on_update=[]),
                            )
                        )
                    si.on_wait = keep
                    changed = True
                out.append(inst)
            if changed:
                try:
                    blk.instructions = out
                except Exception:
                    blk.instructions.clear()
                    blk.instructions.extend(out)
    return ctr


def build_module(t_steps=T, bl=BL, reps=1, streams=STREAMS, blocks=True,
                 idmm="pe"):
    """Build the Bass module (single program, run SPMD on 8 cores).

    reps>1 repeats the whole wavefront (same x, state carried over) for
    slope-timing the device execution under the axon dispatch overhead.
    blocks=True emits each stream's step as one contiguous block (engine
    queues then serve streams phase-shifted instead of head-of-line blocking
    each other); idmm selects PE identity-accumulate vs DVE add for xn + r*hn.
    """
    from contextlib import ExitStack

    import concourse.bass as bass
    import concourse.tile as tile
    from concourse import mybir

    f32 = mybir.dt.float32
    bf16 = mybir.dt.bfloat16
    AF = mybir.ActivationFunctionType
    OP = mybir.AluOpType

    fw = bl // streams
    CW = 776

    nc = bass.Bass()

    x_d = nc.dram_tensor("x", [t_steps, D, bl], bf16, kind="ExternalInput")
    cb_d = nc.dram_tensor("cb", [128, CW], bf16, kind="ExternalInput")
    cf_d = nc.dram_tensor("cf", [128, 8], f32, kind="ExternalInput")
    out_d = nc.dram_tensor("out", [1, bl], f32, kind="ExternalOutput")

    with ExitStack() as ctx:
        tc = ctx.enter_context(tile.TileContext(nc))
        const = ctx.enter_context(tc.tile_pool(name="const", bufs=1))
        work = ctx.enter_context(tc.tile_pool(name="work", bufs=2))
        cpool = ctx.enter_context(tc.tile_pool(name="cpool", bufs=3))
        pzb = ctx.enter_context(tc.tile_pool(name="pzb", bufs=2, space="PSUM"))
        prb = ctx.enter_context(tc.tile_pool(name="prb", bufs=2, space="PSUM"))
        phb = ctx.enter_context(tc.tile_pool(name="phb", bufs=2, space="PSUM"))
        pnb = ctx.enter_context(tc.tile_pool(name="pnb", bufs=2, space="PSUM"))

        # ---- constants in SBUF (two packed tiles, two DMAs) ----
        cb = const.tile([128, CW], bf16, tag="cb")
        nc.sync.dma_start(out=cb, in_=cb_d[:])
        cf = const.tile([128, 8], f32, tag="cf")
        nc.sync.dma_start(out=cf, in_=cf_d[:])

        Wx0_z = cb[0:D, 0:64]
        Wx0_r = cb[0:D, 64:128]
        Wx0_n = cb[0:D, 128:192]
        Wh0_z = cb[0:H, 192:256]
        Wh0_r = cb[0:H, 256:320]
        Wh0_n = cb[0:H, 320:384]
        W1_z = cb[0:128, 384:448]
        W1_r = cb[0:128, 448:512]
        W1_xn = cb[0:H, 512:576]
        W1_hn = cb[H:128, 576:640]
        I128 = cb[0:128, 640:768]
        fcw = cb[H:128, 768:769]
        bz = cf[:, 0:1]
        br = cf[:, 1:2]
        bni = cf[:, 2:3]
        bnh = cf[:, 3:4]
        fcb = cf[0:1, 4:5]

        # ACT warm-up: absorbs the sigmoid/tanh table-load and the cf DMA
        # wait into an instruction with spare wait slots.
        warm = work.tile([128, 8], f32, tag="warm")
        nc.scalar.activation(warm, cf, AF.Sigmoid)
        warm_v = work.tile([128, 8], f32, tag="warm_v")
        nc.vector.tensor_copy(warm_v, cf)

        # Preload all of x: 8 chunk tiles written once each.
        CH = max(1, t_steps // 8)
        x_chunks = []
        for c in range(0, t_steps, CH):
            n_t = min(CH, t_steps - c)
            xc = const.tile([D, n_t, bl], bf16, tag=f"xc{c}")
            nc.sync.dma_start(
                out=xc, in_=x_d[c : c + n_t].rearrange("t d b -> d t b")
            )
            x_chunks.append(xc)

        def xs(s, g):
            s = s % t_steps
            return x_chunks[s // CH][:, s % CH, g]

        SG = [slice(sg * fw, (sg + 1) * fw) for sg in range(streams)]
        PL0 = slice(0, H)
        PL1 = slice(H, 128)

        n_steps = t_steps * reps

        C_prev = []
        for sg in range(streams):
            c0 = cpool.tile([128, fw], bf16, tag=f"c{sg}")
            nc.vector.memset(c0, 0.0)
            C_prev.append(c0)

        mm = nc.tensor.matmul

        def emit_mms(s, sg, zb, rb, hb, nb, l0, l1):
            g, Cp = SG[sg], C_prev[sg]
            if l0:
                mm(rb[PL0, g], lhsT=Wx0_r, rhs=xs(s, g),
                   start=True, stop=False, skip_group_check=True)
                mm(rb[PL0, g], lhsT=Wh0_r, rhs=Cp[PL0, :],
                   start=False, stop=True, skip_group_check=True)
            if l1:
                mm(rb[PL1, g], lhsT=W1_r, rhs=Cp,
                   start=True, stop=True, skip_group_check=True)
            if l0:
                mm(hb[PL0, g], lhsT=Wh0_n, rhs=Cp[PL0, :],
                   start=True, stop=True, skip_group_check=True)
            if l1:
                mm(hb[PL1, g], lhsT=W1_hn, rhs=Cp[PL1, :],
                   start=True, stop=True, skip_group_check=True)
            if l0:
                mm(zb[PL0, g], lhsT=Wx0_z, rhs=xs(s, g),
                   start=True, stop=False, skip_group_check=True)
                mm(zb[PL0, g], lhsT=Wh0_z, rhs=Cp[PL0, :],
                   start=False, stop=True, skip_group_check=True)
            if l1:
                mm(zb[PL1, g], lhsT=W1_z, rhs=Cp,
                   start=True, stop=True, skip_group_check=True)
            if l0:
                mm(nb[PL0, g], lhsT=Wx0_n, rhs=xs(s, g),
                   start=True, stop=idmm != "pe", skip_group_check=True)
            if l1:
                mm(nb[PL1, g], lhsT=W1_xn, rhs=Cp[PL0, :],
                   start=True, stop=idmm != "pe", skip_group_check=True)

        def emit_sigR(sg, rb, pr):
            R = work.tile([128, fw], bf16, tag=f"R{sg}")
            nc.scalar.activation(R[pr, :], rb[pr, SG[sg]], AF.Sigmoid,
                                 bias=br[pr, :])
            return R

        def emit_sigZ(sg, zb, pr):
            Z = work.tile([128, fw], bf16, tag=f"Z{sg}")
            nc.scalar.activation(Z[pr, :], zb[pr, SG[sg]], AF.Sigmoid,
                                 bias=bz[pr, :])
            Zm1 = work.tile([128, fw], bf16, tag=f"Zm1{sg}")
            nc.vector.tensor_scalar_sub(Zm1[pr, :], Z[pr, :], 1.0)
            return Z, Zm1

        def emit_T(sg, hb, R, pr):
            Tt = work.tile([128, fw], bf16, tag=f"T{sg}")
            nc.vector.scalar_tensor_tensor(
                out=Tt[pr, :], in0=hb[pr, SG[sg]], scalar=bnh[pr, :],
                in1=R[pr, :], op0=OP.add, op1=OP.mult)
            return Tt

        def emit_ZH(sg, Z, pr):
            ZH = work.tile([128, fw], bf16, tag=f"ZH{sg}")
            nc.gpsimd.tensor_mul(ZH[pr, :], Z[pr, :], C_prev[sg][pr, :])
            return ZH

        def emit_n(sg, nb, Tt, pr):
            if idmm == "pe":
                mm(nb[pr, SG[sg]], lhsT=I128[pr, pr], rhs=Tt[pr, :],
                   start=False, stop=True, skip_group_check=True)
                nsrc, nbias = nb[pr, SG[sg]], bni[pr, :]
            else:
                U = work.tile([128, fw], bf16, tag=f"U{sg}")
                nc.vector.scalar_tensor_tensor(
                    out=U[pr, :], in0=nb[pr, SG[sg]], scalar=bni[pr, :],
                    in1=Tt[pr, :], op0=OP.add, op1=OP.add)
                nsrc, nbias = U[pr, :], 0.0
            N = work.tile([128, fw], bf16, tag=f"N{sg}")
            nc.scalar.activation(N[pr, :], nsrc, AF.Tanh, bias=nbias)
            return N

        def emit_tail(s, sg, ZH, Zm1, N, pr):
            E3 = work.tile([128, fw], bf16, tag=f"E3{sg}")
            nc.vector.tensor_mul(E3[pr, :], Zm1[pr, :], N[pr, :])
            Cn = cpool.tile([128, fw], bf16, tag=f"c{sg}")
            nc.vector.tensor_sub(Cn[pr, :], ZH[pr, :], E3[pr, :])
            if s == 0:
                # h1(-1) = 0 for layer 1's first step
                nc.gpsimd.memset(Cn[PL1, :], 0.0)
            return Cn

        for s in range(n_steps + 1):
            l0 = s < n_steps
            l1 = s >= 1
            pr = slice(0 if l0 else H, 128 if l1 else H)

            zb = pzb.tile([128, bl], f32, tag="zb")
            rb = prb.tile([128, bl], f32, tag="rb")
            hb = phb.tile([128, bl], f32, tag="hb")
            nb = pnb.tile([128, bl], f32, tag="nb")

            C_new = [None] * streams
            if blocks:
                for sg in range(streams):
                    emit_mms(s, sg, zb, rb, hb, nb, l0, l1)
                    R = emit_sigR(sg, rb, pr)
                    Z, Zm1 = emit_sigZ(sg, zb, pr)
                    Tt = emit_T(sg, hb, R, pr)
                    ZH = emit_ZH(sg, Z, pr)
                    N = emit_n(sg, nb, Tt, pr)
                    C_new[sg] = emit_tail(s, sg, ZH, Zm1, N, pr)
            else:
                for sg in range(streams):
                    emit_mms(s, sg, zb, rb, hb, nb, l0, l1)
                Rs = [emit_sigR(sg, rb, pr) for sg in range(streams)]
                ZZ = [emit_sigZ(sg, zb, pr) for sg in range(streams)]
                Ts = [emit_T(sg, hb, Rs[sg], pr) for sg in range(streams)]
                ZHs = [emit_ZH(sg, ZZ[sg][0], pr) for sg in range(streams)]
                Ns = [emit_n(sg, nb, Ts[sg], pr) for sg in range(streams)]
                for sg in range(streams):
                    C_new[sg] = emit_tail(s, sg, ZHs[sg], ZZ[sg][1], Ns[sg], pr)
            C_prev = C_new

        # final projection: out = fc_w @ h1_final + fc_b  -> [1, bl]
        pfc = pzb.tile([1, bl], f32, tag="zb")
        for sg in range(streams):
            mm(pfc[0:1, SG[sg]], lhsT=fcw, rhs=C_prev[sg][PL1, :],
               start=True, stop=True, skip_group_check=True)
        out_sb = work.tile([1, bl], f32, tag="out")
        nc.scalar.activation(out_sb, pfc, AF.Identity, bias=fcb)
        nc.sync.dma_start(out=out_d[:], in_=out_sb)

    _legalize_sync(nc, mybir)
    return nc


def shard_inputs(inputs, bl=BL, ncores=NCORES, t_steps=T):
    """Host-side prep: transpose/cast/shard full inputs into per-core maps.

    PyTorch/reference GRU gate order in W_ih/W_hh rows is [r | z | n].
    """
    bf = ml_dtypes.bfloat16
    x = np.asarray(inputs["x"], dtype=np.float32)
    xT = np.ascontiguousarray(
        x[: bl * ncores, :t_steps, :].transpose(1, 2, 0)
    ).astype(bf)

    def wT(w, rows):
        w = np.asarray(w, dtype=np.float32)
        return np.ascontiguousarray(w[rows].T).astype(bf)

    R_, Z_, N_ = slice(0, H), slice(H, 2 * H), slice(2 * H, 3 * H)

    CW = 776
    cb = np.zeros((128, CW), dtype=bf)
    cb[0:D, 0:64] = wT(inputs["W_ih0"], Z_)
    cb[0:D, 64:128] = wT(inputs["W_ih0"], R_)
    cb[0:D, 128:192] = wT(inputs["W_ih0"], N_)
    cb[0:H, 192:256] = wT(inputs["W_hh0"], Z_)
    cb[0:H, 256:320] = wT(inputs["W_hh0"], R_)
    cb[0:H, 320:384] = wT(inputs["W_hh0"], N_)
    cb[0:128, 384:448] = np.vstack([wT(inputs["W_ih1"], Z_),
                                    wT(inputs["W_hh1"], Z_)])
    cb[0:128, 448:512] = np.vstack([wT(inputs["W_ih1"], R_),
                                    wT(inputs["W_hh1"], R_)])
    cb[0:H, 512:576] = wT(inputs["W_ih1"], N_)
    cb[H:128, 576:640] = wT(inputs["W_hh1"], N_)
    cb[0:128, 640:768] = np.eye(128, dtype=np.float32).astype(bf)
    cb[H:128, 768] = np.asarray(inputs["fc_w"], dtype=np.float32).reshape(H).astype(bf)

    cf = np.zeros((128, 8), dtype=np.float32)
    for l, (bi_k, bh_k) in enumerate((("b_ih0", "b_hh0"), ("b_ih1", "b_hh1"))):
        bi = np.asarray(inputs[bi_k], dtype=np.float32)
        bh = np.asarray(inputs[bh_k], dtype=np.float32)
        p = slice(l * H, (l + 1) * H)
        cf[p, 0] = bi[Z_] + bh[Z_]
        cf[p, 1] = bi[R_] + bh[R_]
        cf[p, 2] = bi[N_]
        cf[p, 3] = bh[N_]
    cf[0, 4] = np.asarray(inputs["fc_b"], dtype=np.float32).reshape(())

    shared = {"cb": cb, "cf": cf}

    in_maps = []
    for c in range(ncores):
        m = dict(shared)
        m["x"] = np.ascontiguousarray(xT[:, :, c * bl : (c + 1) * bl])
        in_maps.append(m)
    return in_maps


def kernel(**inputs):
    from concourse import bass_utils

    if "nc" not in _CACHE:
        _CACHE["nc"] = build_module()
    nc = _CACHE["nc"]
    in_maps = shard_inputs(inputs)
    res = bass_utils.run_bass_kernel_spmd(nc, in_maps, core_ids=list(range(NCORES)))
    out = np.concatenate([r["out"].reshape(BL) for r in res.results])
    return out.astype(np.float32)
